# revision 1
# baseline (speedup 1.0000x reference)
"""Binarized 3x3 conv (N=32, C=256, H=W=56, pad=1, stride=1) for 8 TRN2 NeuronCores.

Strategy
--------
- Data-parallel over batch: 4 images per core, weight replicated.
- sign-binarized values (+-1) are exact in fp8e4; products are +-1 and the
  conv accumulation (<= 2304 terms) is exact in fp32 PSUM, so the whole
  computation is bit-exact vs the fp32 reference.
- The 3x3 conv is decomposed into 9 shifted matmuls accumulated in PSUM:
  out[co, h, w] += W[co, ci, dh, dw] * x[ci, h+dh, w+dw].
- The image is stored in SBUF zero-padded to 58 rows x 57 cols: one left pad
  column per row; a row's RIGHT pad is the NEXT row's left pad (both zero),
  so 57-wide rows suffice.  Each of the 9 shifted matmuls is one contiguous
  456-wide window (8 out rows x 57) with a single garbage column (w=56)
  dropped on evacuation.  456 vs the previous 464 saves 1.7% PE time.
- Contraction K = Cin = 256 = 2x128 runs in one pass with fp8 DoubleRow
  (lhsT/rhs get a [128, 2, F] access pattern; PE does 2 MACs/cell/cycle).
- Measured on this HW: matmul = 193ns per 464-row AP (1.2GHz-class rate),
  marginal LDWEIGHTS cost ~3ns (fully pipelined) -- so loop structure is
  free to split ki-sweeps into row-groups.
- Startup: image 0 is loaded in 6 small row bands (one DMA each, both
  channel halves per transfer) interleaved with j0-half weight chunks;
  the PE sweeps image 0 j=0 in per-row groups gated on bands, so the first
  matmul starts ~4-5us in instead of waiting for the whole image.
- Steady state: ki-outer / r-inner sweeps with the ki=0 sweep split
  {r0-3}/{r4-6} so a new group's first matmul doesn't wait (via Tile's
  wait coalescing on the in-order PE stream) for ALL 7 PSUM banks of the
  previous group to evacuate.
- Binarize is ScalarE Sign (bias=-1e-30 maps exact 0 -> -1 like the
  reference).
- Outputs are integers in [-2304, 2304]: evacuate PSUM as int16 (exact) and
  DMA half the bytes; the host upcasts back to fp32.  Evacuation alternates
  ACT/DVE per bank; the final group splits each bank across both engines
  and stores on the idle HWDGE rings to shorten the kernel tail.
"""

import os

import numpy as np

C = 256
H = W = 56
WP = W + 1                      # window cols per output row (56 data + 1 junk)
NROWS = H + 2                   # padded rows (58)
CPAD = 64                       # padded row width; left pad col 0, data 1..56,
                                # zeros 57..63.  i-plane stride 64 and row
                                # stride 128 both satisfy the DoubleRow
                                # %16 AP step rule, and the row-interleaved
                                # [row][i][col] layout keeps every matmul
                                # rhs AP's bounding box tight, so Tile's
                                # interval-based dependency tracking gates
                                # each matmul only on the row band it reads.
R = 8                           # output rows per PSUM block
NBLK = 7                        # row blocks (7*8 = 56)
NWIN = R * WP                   # matmul free dim: 456

# image-0 row bands (fast start); other images load in 2 chunks
IMG0_BANDS = [(0, 10), (10, 18), (18, 26), (26, 34), (34, 44), (44, 56)]
STEADY_BANDS = [(0, 28), (28, 56)]
# image-0 j=0 PE groups: per-row, gated on bands; elsewhere split ki=0 only
IMG0_GROUPS = [[0], [1], [2], [3], [4], [5, 6]]


def build_nc(img_per_core=4, mode="fp8", debug=False, *, xp_bufs=3,
             reps=1, steady_split=True, img0_fine=True, store_ring="gpsimd"):
    import contextlib

    import concourse.bacc as bacc
    import concourse.mybir as mybir
    from concourse import tile
    from concourse.tile_rust import add_dep_helper

    assert mode == "fp8", "v2 kernel only implements fp8 DoubleRow"
    f32 = mybir.dt.float32
    i16 = mybir.dt.int16
    cdt = mybir.dt.float8e4
    pm = mybir.MatmulPerfMode.DoubleRow

    nc = bacc.Bacc("TRN2", target_bir_lowering=False, debug=debug)
    x = nc.dram_tensor("x", [img_per_core, C, H, W], f32, kind="ExternalInput")
    wt = nc.dram_tensor("wt", [128, 2, 2, 9, 128], f32,
                        kind="ExternalInput")
    y = nc.dram_tensor("y", [img_per_core, C, H, W], i16, kind="ExternalOutput")

    # dram view: [img, 128, 2, H*W] so one DMA moves both channel halves
    x2 = x[:, :, :, :].rearrange("n (i p) h w -> n p i (h w)", i=2)
    yf = y[:, :, :, :].rearrange("n c h w -> n c (h w)")

    with tile.TileContext(nc) as tc:
        with tc.tile_pool(name="wp", bufs=1) as wp, \
             tc.tile_pool(name="xsp", bufs=3) as xsp, \
             tc.tile_pool(name="xqp", bufs=1) as xqp, \
             tc.tile_pool(name="op", bufs=4) as op, \
             tc.tile_pool(name="pp", bufs=8, space="PSUM") as pp:

            # Sign bias: tiny negative so exact-0 inputs binarize to -1 like
            # the reference's (x <= 0 -> -1).
            bias_t = wp.tile([128, 1], f32, tag="bias")
            nc.vector.memset(bias_t[:, :], -1e-30)
            neg_eps = bias_t[:, 0:1]

            # Dummy sign fired first so the 1.3us LoadActFuncSet table load
            # runs at t=0, off the critical path of the first real sign.
            warm = wp.tile([128, 1], cdt, tag="warm")
            nc.scalar.sign(warm[:, :], bias_t[:, :], bias=neg_eps)

            # --- weights: fp32 [128, 2, 9, 256]; binarize per chunk.
            # j=0 columns first (in 3 ki-chunks, interleaved with image-0
            # bands below), j=1 as one transfer signed in two pieces.
            wstage = wp.tile([128, 2, 2, 9, 128], f32, tag="wstage")
            w8 = wp.tile([128, 2, 2, 9, 128], cdt, tag="w8")

            def load_w(ks, js, nsign=1):
                nc.sync.dma_start(wstage[:, js, :, ks, :], wt[:, js, :, ks, :])
                if nsign == 1:
                    nc.scalar.sign(w8[:, js, :, ks, :],
                                   wstage[:, js, :, ks, :], bias=neg_eps)
                else:
                    mid = (ks.start + ks.stop) // 2
                    for s in (slice(ks.start, mid), slice(mid, ks.stop)):
                        nc.scalar.sign(w8[:, js, :, s, :],
                                       wstage[:, js, :, s, :], bias=neg_eps)

            # --- persistent padded input tiles; zero the pad cells once:
            # [0,58) = top pad row + row1's left pad; left-pad col of rows
            # 2..56; [3249,3312) = bottom pad row + tail (read by the
            # junk column and the very last window element).
            xps = []
            for b in range(xp_bufs):
                xpq = xqp.tile([128, NROWS, 2, CPAD], cdt, tag=f"xp{b}",
                               name=f"xp{b}")
                xps.append(xpq)
                nc.vector.memset(xpq[:, 0, :, :], 0)       # top pad row
                nc.vector.memset(xpq[:, 57, :, :], 0)      # bottom pad row
                nc.vector.memset(xpq[:, 1:57, :, 0:1], 0)  # left pad col
                nc.vector.memset(xpq[:, 1:57, :, 57:64], 0)  # right pad cols

            # Chain matmuls in emission order (ordering-only deps) so the
            # scheduler keeps runs of identical stationary weights intact
            # for _dedup_ldweights.
            prev_mm = [None]

            def chain(mm):
                if prev_mm[0] is not None:
                    add_dep_helper(mm.ins, prev_mm[0].ins, sync=False,
                                   reason="pe emission order")
                prev_mm[0] = mm

            def band_dma(img, rs, re, ring):
                n = re - rs
                xstage = xsp.tile([128, 2, n * W], f32, tag=f"xs{n}")
                ring.dma_start(xstage[:, :, :],
                               x2[img, :, :, rs * W:re * W])
                return xstage

            def band_sign(xp, xstage, rs, re):
                for i in range(2):
                    src = xstage[:, i, :].rearrange("p (h w) -> p h w", w=W)
                    dst = xp[:, rs + 1:re + 1, i, 1:1 + W]
                    nc.scalar.sign(dst, src, bias=neg_eps)

            def load_band(img, xp, rs, re, ring):
                band_sign(xp, band_dma(img, rs, re, ring), rs, re)

            def sweep(xp, pts, blocks, j, rgroup):
                """ki-outer, r-inner matmul sweep over rgroup's banks."""
                for ki in range(9):
                    dh, dw = ki // 3 - 1, ki % 3 - 1
                    lhsT = w8[:, j, :, ki, :]
                    for r in rgroup:
                        row0, nr = blocks[r]
                        rs = row0 + dh + 1
                        rhs = xp[:, rs:rs + nr, :, dw + 1:dw + 1 + WP] \
                            .rearrange("p r i c -> p i r c")
                        chain(nc.tensor.matmul(
                            pts[r][:, :, :], lhsT, rhs,
                            start=(ki == 0), stop=(ki == 8), perf_mode=pm))

            def evac(pts, ost, blocks, r, eng):
                """PSUM bank r -> int16 SBUF rows, dropping junk col 56."""
                row0, nr = blocks[r]
                src = pts[r][:, :, 0:W]
                dst = ost[:, row0:row0 + nr, :]
                if eng == "act":
                    nc.scalar.copy(dst, src)
                elif eng == "dve":
                    nc.vector.tensor_copy(dst, src)
                else:                # split across both engines
                    h = nr // 2
                    nc.scalar.copy(dst[:, 0:h, :], pts[r][:, 0:h, 0:W])
                    nc.vector.tensor_copy(dst[:, h:nr, :],
                                          pts[r][:, h:nr, 0:W])

            def store(img, j, ost, a, b, eng):
                ostf = ost[:, :, :].rearrange("p h w -> p (h w)")
                eng.dma_start(yf[img, j * 128:(j + 1) * 128, a * W:b * W],
                              ostf[:, a * W:b * W])

            # reps>1 is a benchmarking aid: repeat the whole pipeline inside
            # one NEFF via a dynamic loop.
            loop = tc.For_i(0, reps, 1) if reps > 1 else contextlib.nullcontext()
            with loop:
              for img in range(img_per_core):
                xp = xps[img % len(xps)]
                first = img == 0 and img0_fine
                if first:
                    # interleave j0 weight chunks with the early bands so
                    # the r0 ki-sweep is fed as it runs; band 1's signs are
                    # emitted after the late weight chunks so ACT binarizes
                    # the ki5-8 weights (needed sooner) first.
                    load_w(slice(0, 1), 0)
                    load_band(img, xp, *IMG0_BANDS[0], nc.sync)
                    load_w(slice(1, 3), 0)
                    xs1 = band_dma(img, *IMG0_BANDS[1], nc.sync)
                    load_w(slice(3, 6), 0)
                    load_w(slice(6, 9), 0)
                    band_sign(xp, xs1, *IMG0_BANDS[1])
                    for band in IMG0_BANDS[2:]:
                        load_band(img, xp, *band, nc.sync)
                    load_w(slice(0, 9), 1, nsign=2)
                else:
                    for band in STEADY_BANDS:
                        load_band(img, xp, *band, nc.sync)

                st_eng = {"gpsimd": nc.gpsimd, "scalar": nc.scalar,
                          "sync": nc.sync}[store_ring]
                for j in range(2):
                    ost = op.tile([128, H, W], i16, tag="ost",
                                  name=f"ost{img}_{j}")
                    last = img == img_per_core - 1 and j == 1
                    if last:
                        # final block alone in the last group; its evac is
                        # split ACT+DVE and its store rides the idle sync
                        # HWDGE ring, so the tail is one evac + one store.
                        blocks = [(R * r, R) for r in range(NBLK)]
                        groups = [[0, 1, 2, 3], [4, 5], [6]]
                        evacs = ["act", "dve", "act", "dve",
                                 "act", "dve", "split"]
                        pieces = {1: (0, 16, st_eng), 3: (16, 32, st_eng),
                                  5: (32, 48, st_eng), 6: (48, 56, nc.sync)}
                    else:
                        blocks = [(R * r, R) for r in range(NBLK)]
                        if first and j == 0:
                            groups = IMG0_GROUPS
                        elif steady_split:
                            groups = [[0, 1, 2, 3], [4, 5, 6]]
                        else:
                            groups = [[0, 1, 2, 3, 4, 5, 6]]
                        evacs = ["act" if r % 2 == 0 else "dve"
                                 for r in range(NBLK)]
                        pieces = {1: (0, 16, st_eng), 3: (16, 32, st_eng),
                                  5: (32, 48, st_eng), 6: (48, 56, st_eng)}
                    pts = [pp.tile([128, nr, WP], f32, tag="pt",
                                   name=f"pt{img}_{j}_{r}")
                           for r, (row0, nr) in enumerate(blocks)]
                    for g in groups:
                        sweep(xp, pts, blocks, j, g)
                        for r in g:
                            evac(pts, ost, blocks, r, evacs[r])
                            if r in pieces:
                                a, b, eng = pieces[r]
                                store(img, j, ost, a, b, eng)

    _dedup_ldweights(nc)
    # bacc's move_matmul_waits_to_ldweights would hoist every matmul's
    # psum-slot wait onto the (deduped) LDW at the head of its weight
    # run, stalling the whole run until all slots are free.
    nc.move_matmul_waits_to_ldweights = lambda: None
    nc.compile()
    return nc


def _dedup_ldweights(nc):
    """Remove InstLdweights that reload the exact weights already resident.

    The Tile layer emits one LDWEIGHTS per matmul even when consecutive
    matmuls share the stationary operand.  Deleting a duplicate is safe:
    the PE keeps loaded weights until the next LDW, and
    InstMatmult(ldweights=False) does not self-load.  Any sem waits that
    Tile parked on a deleted LDW are merged into the next PE instruction
    (waits may only move later in the stream, which preserves ordering).
    """
    import bass_rust

    n_del = 0
    for blk in nc.main_func.blocks:
        out = []
        last_key = None
        pending = []
        for inst in blk.instructions:
            tn = type(inst).__name__
            if tn == "InstLdweights":
                key = (str(inst.ins[0]), str(inst.perf_mode),
                       str(inst.is_transpose), str(inst.tile_position))
                if key == last_key:
                    si = inst.sync_info
                    if si is not None:
                        ups = list(si.on_update)
                        assert not ups, f"dup LDW {inst.name} carries updates"
                        pending.extend(list(si.on_wait))
                    n_del += 1
                    continue
                last_key = key
            if tn in ("InstLdweights", "InstMatmult") and pending:
                si = inst.sync_info
                waits = list(si.on_wait) if si is not None else []
                ups = list(si.on_update) if si is not None else []
                merged = {}
                for w in waits + pending:
                    k2 = (w.id, getattr(w, "wait_mode", None))
                    prev = merged.get(k2)
                    if prev is None or (w.wait_value or 0) > (prev.wait_value or 0):
                        merged[k2] = w
                inst.sync_info = bass_rust.SyncInfo(
                    on_wait=list(merged.values()), on_update=ups)
                pending = []
            out.append(inst)
        assert not pending, "dangling waits from deleted LDW at block end"
        blk.instructions = out
    return n_del


def prep_weight(weight: np.ndarray, swi: bool = False) -> np.ndarray:
    # [co, ci, kh, kw] -> wt[p, j, i, k, c] = weight[j*128+c, i*128+p, k]
    w5 = weight.reshape(2, 128, 2, 128, 9)      # [j, c, i, p, k]
    return np.ascontiguousarray(np.transpose(w5, (3, 0, 2, 4, 1)))


def run(x, weight, n_cores=8, mode="fp8", trace=False, **kw):
    from concourse.bass_utils import run_bass_kernel_spmd

    x = np.ascontiguousarray(np.asarray(x, dtype=np.float32))
    weight = np.ascontiguousarray(np.asarray(weight, dtype=np.float32))
    n = x.shape[0]
    per = n // n_cores
    wt = prep_weight(weight)
    nc = build_nc(img_per_core=per, mode=mode, **kw)
    in_maps = [
        {"x": x[c * per:(c + 1) * per], "wt": wt} for c in range(n_cores)
    ]
    res = run_bass_kernel_spmd(
        nc, in_maps, core_ids=list(range(n_cores)), trace=trace)
    y = np.concatenate([r["y"] for r in res.results], axis=0)
    if y.dtype != np.float32:
        y = y.astype(np.float32)
    return y, res


def kernel(x, weight):
    y, _ = run(x, weight, mode=os.environ.get("BINCONV_MODE", "fp8"))
    return y



# revision 4
# speedup vs baseline: 1.0133x; 1.0133x over previous
"""Binarized 3x3 conv (N=32, C=256, H=W=56, pad=1, stride=1) for 8 TRN2 NeuronCores.

Strategy (v3)
-------------
- Data-parallel over batch: 4 images per core, weight replicated.
- sign-binarized values (+-1) are exact in fp8e4; products are +-1 and the
  conv accumulation (<= 2304 terms) is exact in fp32 PSUM, so the whole
  computation is bit-exact vs the fp32 reference.
- The 3x3 conv is decomposed into 9 shifted matmuls accumulated in PSUM:
  out[co, h, w] += W[co, ci, dh, dw] * x[ci, h+dh, w+dw].
- The image is stored in SBUF zero-padded to 58 rows x 64 cols (left pad
  col 0, data 1..56, zero 57..63).  Each of the 9 shifted matmuls is a
  rank-4 AP window [p, i, 8 rows, 56 cols] -- exactly the useful outputs,
  no junk column (448 free vs v2's 456: 1.75% less PE time).
- Contraction K = Cin = 256 = 2x128 runs in one pass with fp8 DoubleRow
  (lhsT/rhs get a [128, 2, ...] access pattern; PE does 2 MACs/cell/cycle).
- Engine split (v3): ACT does binarize ONLY, so the next image's sign is
  never head-of-line blocked behind the previous image's PSUM evacuation
  (which used to stall the PE ~3.5us at every image boundary and reset the
  PE p-state ramp).  Evacuations alternate DVE/Pool; stores ride the Pool
  SWDGE ring; weight loads ride the ACT HWDGE ring so x-band loads have
  the SP HWDGE ring to themselves (startup: first matmul ~2.5us instead
  of ~6.6us).
- xp_bufs=4 (one padded-input buffer per in-flight image) so the reps
  loop boundary pipelines: the next rep's first loads have no WAR wait.
- Binarize is ScalarE Sign (bias=-1e-30 maps exact 0 -> -1 like the
  reference).
- Outputs are integers in [-2304, 2304]: evacuate PSUM as int16 (exact) and
  DMA half the bytes; the host upcasts back to fp32.
"""

import os

import numpy as np

C = 256
H = W = 56
WP = W                          # window cols per output row (56, no junk)
NROWS = H + 2                   # padded rows (58)
CPAD = 64                       # padded row width; left pad col 0, data 1..56,
                                # zeros 57..63.  i-plane stride 64 and row
                                # stride 128 both satisfy the DoubleRow
                                # %16 AP step rule, and the row-interleaved
                                # [row][i][col] layout keeps every matmul
                                # rhs AP's bounding box tight, so Tile's
                                # interval-based dependency tracking gates
                                # each matmul only on the row band it reads.
R = 8                           # output rows per PSUM block
NBLK = 7                        # row blocks (7*8 = 56)
NWIN = R * WP                   # matmul free dim: 448

# image-0 row bands (fast start); other images load in 2 chunks
IMG0_BANDS = [(0, 10), (10, 18), (18, 26), (26, 34), (34, 44), (44, 56)]
STEADY_BANDS = [(0, 28), (28, 56)]
# image-0 j=0 PE groups: per-row, gated on bands; elsewhere split ki=0 only
IMG0_GROUPS = [[0], [1], [2], [3], [4], [5, 6]]


def build_nc(img_per_core=4, mode="fp8", debug=False, *, xp_bufs=4,
             reps=1, steady_split=True, img0_fine=True, store_ring="gpsimd",
             w_ring="sync", evac="act_light", prefetch_signs=True):
    import contextlib

    import concourse.bacc as bacc
    import concourse.mybir as mybir
    from concourse import tile
    from concourse.tile_rust import add_dep_helper

    assert mode == "fp8", "kernel only implements fp8 DoubleRow"
    f32 = mybir.dt.float32
    i16 = mybir.dt.int16
    cdt = mybir.dt.float8e4
    pm = mybir.MatmulPerfMode.DoubleRow

    nc = bacc.Bacc("TRN2", target_bir_lowering=False, debug=debug)
    x = nc.dram_tensor("x", [img_per_core, C, H, W], f32, kind="ExternalInput")
    wt = nc.dram_tensor("wt", [128, 2, 2, 9, 128], f32,
                        kind="ExternalInput")
    y = nc.dram_tensor("y", [img_per_core, C, H, W], i16, kind="ExternalOutput")

    # dram view: [img, 128, 2, H*W] so one DMA moves both channel halves
    x2 = x[:, :, :, :].rearrange("n (i p) h w -> n p i (h w)", i=2)
    yf = y[:, :, :, :].rearrange("n c h w -> n c (h w)")

    with tile.TileContext(nc) as tc:
        with tc.tile_pool(name="wp", bufs=1) as wp, \
             tc.tile_pool(name="xsp", bufs=3) as xsp, \
             tc.tile_pool(name="xqp", bufs=1) as xqp, \
             tc.tile_pool(name="op", bufs=4) as op, \
             tc.tile_pool(name="pp", bufs=8, space="PSUM") as pp:

            # Sign bias: tiny negative so exact-0 inputs binarize to -1 like
            # the reference's (x <= 0 -> -1).
            bias_t = wp.tile([128, 1], f32, tag="bias")
            nc.vector.memset(bias_t[:, :], -1e-30)
            neg_eps = bias_t[:, 0:1]

            # Dummy sign fired first so the 1.3us LoadActFuncSet table load
            # runs at t=0, off the critical path of the first real sign.
            warm = wp.tile([128, 1], cdt, tag="warm")
            nc.scalar.sign(warm[:, :], bias_t[:, :], bias=neg_eps)

            wring = {"scalar": nc.scalar, "sync": nc.sync}[w_ring]

            # --- weights: fp32 [128, 2, 9, 256]; binarize per chunk.
            # j=0 columns first (in 3 ki-chunks, interleaved with image-0
            # bands below), j=1 as one transfer signed in two pieces.
            wstage = wp.tile([128, 2, 2, 9, 128], f32, tag="wstage")
            w8 = wp.tile([128, 2, 2, 9, 128], cdt, tag="w8")

            def load_w(ks, js, nsign=1):
                wring.dma_start(wstage[:, js, :, ks, :], wt[:, js, :, ks, :])
                if nsign == 1:
                    nc.scalar.sign(w8[:, js, :, ks, :],
                                   wstage[:, js, :, ks, :], bias=neg_eps)
                else:
                    mid = (ks.start + ks.stop) // 2
                    for s in (slice(ks.start, mid), slice(mid, ks.stop)):
                        nc.scalar.sign(w8[:, js, :, s, :],
                                       wstage[:, js, :, s, :], bias=neg_eps)

            # --- persistent padded input tiles; zero the pad cells once:
            # top/bottom pad rows, left pad col, right pad cols.
            xps = []
            for b in range(xp_bufs):
                xpq = xqp.tile([128, NROWS, 2, CPAD], cdt, tag=f"xp{b}",
                               name=f"xp{b}")
                xps.append(xpq)
                nc.vector.memset(xpq[:, 0, :, :], 0)       # top pad row
                nc.vector.memset(xpq[:, 57, :, :], 0)      # bottom pad row
                nc.vector.memset(xpq[:, 1:57, :, 0:1], 0)  # left pad col
                nc.vector.memset(xpq[:, 1:57, :, 57:64], 0)  # right pad cols

            # Chain matmuls in emission order (ordering-only deps) so the
            # scheduler keeps runs of identical stationary weights intact
            # for _dedup_ldweights.
            prev_mm = [None]

            def chain(mm):
                if prev_mm[0] is not None:
                    add_dep_helper(mm.ins, prev_mm[0].ins, sync=False,
                                   reason="pe emission order")
                prev_mm[0] = mm

            def band_dma(img, rs, re, ring):
                n = re - rs
                xstage = xsp.tile([128, 2, n * W], f32, tag=f"xs{n}")
                ring.dma_start(xstage[:, :, :],
                               x2[img, :, :, rs * W:re * W])
                return xstage

            def band_sign(xp, xstage, rs, re):
                for i in range(2):
                    src = xstage[:, i, :].rearrange("p (h w) -> p h w", w=W)
                    dst = xp[:, rs + 1:re + 1, i, 1:1 + W]
                    nc.scalar.sign(dst, src, bias=neg_eps)

            def load_band(img, xp, rs, re, ring):
                band_sign(xp, band_dma(img, rs, re, ring), rs, re)

            def sweep(xp, pts, blocks, j, rgroup):
                """ki-outer, r-inner matmul sweep over rgroup's banks."""
                for ki in range(9):
                    dh, dw = ki // 3 - 1, ki % 3 - 1
                    lhsT = w8[:, j, :, ki, :]
                    for r in rgroup:
                        row0, nr = blocks[r]
                        rs = row0 + dh + 1
                        rhs = xp[:, rs:rs + nr, :, dw + 1:dw + 1 + WP] \
                            .rearrange("p r i c -> p i r c")
                        chain(nc.tensor.matmul(
                            pts[r][:, :, :], lhsT, rhs,
                            start=(ki == 0), stop=(ki == 8), perf_mode=pm))

            def evac(pts, ost, blocks, r, eng):
                """PSUM bank r -> int16 SBUF rows."""
                row0, nr = blocks[r]
                src = pts[r][:, :, :]
                dst = ost[:, row0:row0 + nr, :]
                if eng == "act":
                    nc.scalar.copy(dst, src)
                elif eng == "dve":
                    nc.vector.tensor_copy(dst, src)
                elif eng == "pool":
                    nc.gpsimd.tensor_copy(dst, src)
                else:                # split across engines for the tail
                    h = nr // 2
                    nc.scalar.copy(dst[:, 0:h, :], pts[r][:, 0:h, :])
                    nc.vector.tensor_copy(dst[:, h:nr, :],
                                          pts[r][:, h:nr, :])

            def store(img, j, ost, a, b, eng):
                ostf = ost[:, :, :].rearrange("p h w -> p (h w)")
                eng.dma_start(yf[img, j * 128:(j + 1) * 128, a * W:b * W],
                              ostf[:, a * W:b * W])

            if evac == "act_light":
                # ACT carries only 2 of 7 banks so binarize (ACT-only) is
                # never far behind; DVE takes the rest.
                steady_evacs = ["act", "dve", "dve", "act",
                                "dve", "dve", "dve"]
            else:
                steady_evacs = ["act" if r % 2 == 0 else "dve"
                                for r in range(NBLK)]

            # reps>1 is a benchmarking aid: repeat the whole pipeline inside
            # one NEFF via a dynamic loop.
            loop = tc.For_i(0, reps, 1) if reps > 1 else contextlib.nullcontext()
            with loop:
              for img in range(img_per_core):
                xp = xps[img % len(xps)]
                first = img == 0 and img0_fine
                if first:
                    # interleave j0 weight chunks with the early bands so
                    # the r0 ki-sweep is fed as it runs; band 1's signs are
                    # emitted after the late weight chunks so ACT binarizes
                    # the ki5-8 weights (needed sooner) first.
                    load_w(slice(0, 1), 0)
                    load_band(img, xp, *IMG0_BANDS[0], nc.sync)
                    load_w(slice(1, 3), 0)
                    xs1 = band_dma(img, *IMG0_BANDS[1], nc.sync)
                    load_w(slice(3, 6), 0)
                    load_w(slice(6, 9), 0)
                    band_sign(xp, xs1, *IMG0_BANDS[1])
                    for band in IMG0_BANDS[2:]:
                        load_band(img, xp, *band, nc.sync)
                    load_w(slice(0, 9), 1, nsign=2)
                elif not prefetch_signs:
                    for band in STEADY_BANDS:
                        load_band(img, xp, *band, nc.sync)

                st_eng = {"gpsimd": nc.gpsimd, "scalar": nc.scalar,
                          "sync": nc.sync}[store_ring]
                for j in range(2):
                    ost = op.tile([128, H, W], i16, tag="ost",
                                  name=f"ost{img}_{j}")
                    last = img == img_per_core - 1 and j == 1
                    blocks = [(R * r, R) for r in range(NBLK)]
                    if last:
                        # final block alone in the last group; its evac is
                        # split ACT+DVE (both idle by then) and its two
                        # store halves ride the idle ACT HWDGE ring, so the
                        # tail is one short evac + two parallel stores.
                        groups = [[0, 1, 2, 3], [4, 5], [6]]
                        evacs = steady_evacs[:6] + ["split"]
                        pieces = {1: [(0, 16, st_eng)],
                                  3: [(16, 32, st_eng)],
                                  5: [(32, 48, st_eng)],
                                  6: [(48, 52, nc.scalar),
                                      (52, 56, nc.scalar)]}
                    else:
                        if first and j == 0:
                            groups = IMG0_GROUPS
                        elif steady_split:
                            groups = [[0, 1, 2, 3], [4, 5, 6]]
                        else:
                            groups = [[0, 1, 2, 3, 4, 5, 6]]
                        evacs = steady_evacs
                        pieces = {1: [(0, 16, st_eng)],
                                  3: [(16, 32, st_eng)],
                                  5: [(32, 48, st_eng)],
                                  6: [(48, 56, st_eng)]}
                    pts = [pp.tile([128, nr, WP], f32, tag="pt",
                                   name=f"pt{img}_{j}_{r}")
                           for r, (row0, nr) in enumerate(blocks)]
                    for gi, g in enumerate(groups):
                        sweep(xp, pts, blocks, j, g)
                        for r in g:
                            evac(pts, ost, blocks, r, evacs[r])
                            for a, b, eng in pieces.get(r, ()):
                                store(img, j, ost, a, b, eng)
                        # Emit the next image's loads + signs between j=1's
                        # first and second group so ACT's in-order stream
                        # interleaves them after this image's early evacs
                        # (all later j=1 evacs below are DVE-only, so a
                        # sign waiting on DMA can't head-of-line block a
                        # PSUM-bank release).
                        if prefetch_signs and j == 1 and gi == 0 \
                                and img + 1 < img_per_core:
                            nxp = xps[(img + 1) % len(xps)]
                            for band in STEADY_BANDS:
                                load_band(img + 1, nxp, *band, nc.sync)

    _dedup_ldweights(nc)
    # bacc's move_matmul_waits_to_ldweights would hoist every matmul's
    # psum-slot wait onto the (deduped) LDW at the head of its weight
    # run, stalling the whole run until all slots are free.
    nc.move_matmul_waits_to_ldweights = lambda: None
    nc.compile()
    return nc


def _dedup_ldweights(nc):
    """Remove InstLdweights that reload the exact weights already resident.

    The Tile layer emits one LDWEIGHTS per matmul even when consecutive
    matmuls share the stationary operand.  Deleting a duplicate is safe:
    the PE keeps loaded weights until the next LDW, and
    InstMatmult(ldweights=False) does not self-load.  Any sem waits that
    Tile parked on a deleted LDW are merged into the next PE instruction
    (waits may only move later in the stream, which preserves ordering).
    """
    import bass_rust

    n_del = 0
    for blk in nc.main_func.blocks:
        out = []
        last_key = None
        pending = []
        for inst in blk.instructions:
            tn = type(inst).__name__
            if tn == "InstLdweights":
                key = (str(inst.ins[0]), str(inst.perf_mode),
                       str(inst.is_transpose), str(inst.tile_position))
                if key == last_key:
                    si = inst.sync_info
                    if si is not None:
                        ups = list(si.on_update)
                        assert not ups, f"dup LDW {inst.name} carries updates"
                        pending.extend(list(si.on_wait))
                    n_del += 1
                    continue
                last_key = key
            if tn in ("InstLdweights", "InstMatmult") and pending:
                si = inst.sync_info
                waits = list(si.on_wait) if si is not None else []
                ups = list(si.on_update) if si is not None else []
                merged = {}
                for w in waits + pending:
                    k2 = (w.id, getattr(w, "wait_mode", None))
                    prev = merged.get(k2)
                    if prev is None or (w.wait_value or 0) > (prev.wait_value or 0):
                        merged[k2] = w
                inst.sync_info = bass_rust.SyncInfo(
                    on_wait=list(merged.values()), on_update=ups)
                pending = []
            out.append(inst)
        assert not pending, "dangling waits from deleted LDW at block end"
        blk.instructions = out
    return n_del


def prep_weight(weight: np.ndarray, swi: bool = False) -> np.ndarray:
    # [co, ci, kh, kw] -> wt[p, j, i, k, c] = weight[j*128+c, i*128+p, k]
    w5 = weight.reshape(2, 128, 2, 128, 9)      # [j, c, i, p, k]
    return np.ascontiguousarray(np.transpose(w5, (3, 0, 2, 4, 1)))


def run(x, weight, n_cores=8, mode="fp8", trace=False, **kw):
    from concourse.bass_utils import run_bass_kernel_spmd

    x = np.ascontiguousarray(np.asarray(x, dtype=np.float32))
    weight = np.ascontiguousarray(np.asarray(weight, dtype=np.float32))
    n = x.shape[0]
    per = n // n_cores
    wt = prep_weight(weight)
    nc = build_nc(img_per_core=per, mode=mode, **kw)
    in_maps = [
        {"x": x[c * per:(c + 1) * per], "wt": wt} for c in range(n_cores)
    ]
    res = run_bass_kernel_spmd(
        nc, in_maps, core_ids=list(range(n_cores)), trace=trace)
    y = np.concatenate([r["y"] for r in res.results], axis=0)
    if y.dtype != np.float32:
        y = y.astype(np.float32)
    return y, res


def kernel(x, weight):
    y, _ = run(x, weight, mode=os.environ.get("BINCONV_MODE", "fp8"))
    return y
